# revision 1
# baseline (speedup 1.0000x reference)
"""Causal self-attention (B=4, T=2048, C=1024, H=16) on 8 trn2 NeuronCores.

Sharding: core = (batch b, head-half s).  Each core computes q/k/v
projections for its 8 heads (weights pre-sliced/transposed on host),
causal flash-style attention with transposed score tiles, and a partial
(row-sharded) c_proj.  Host gather sums the two partials per batch.

Device data layout (all fp32):
  xT    [1024, 2048]  x[b].T                      (in-ch on partitions)
  wqkT  [1024, 1024]  [Wq_local | Wk_local].T     (in-ch on partitions)
  bqk   [128, 8]      q/k bias, per out-ch block
  wvT   [1024, 512]   Wv_local.T
  wpT   [512, 1024]   Wproj[:, local].T
  bpj   [128, 8]      bproj + bv@WprojT (folded), half of it per core
  zT    [1024, 2048]  partial output, transposed
"""

import os
import sys

sys.path.insert(0, "/opt/trn_rl_repo")

import numpy as np

B, T, C, H = 4, 2048, 1024, 16
D = 64          # head dim
NH = 8          # heads per core
LC = NH * D     # local channels = 512
P = 128
QT = 512        # query tile (also matmul moving free dim)
NQT = T // QT   # 4
NKB = T // P    # 16 key blocks
IC = C // P     # 8 input-channel blocks

# matmul input dtype: float32r = full-rate PE mode (reduced precision),
# float32 = exact but 4x slower.
MM_DT = os.environ.get("BASS_ATTN_MM_DT", "float32r")

_nc_cache = {}


def _build_nc():
    from contextlib import ExitStack

    import concourse.bass as bass  # noqa: F401
    import concourse.mybir as mybir
    from concourse import bacc, tile

    f32 = mybir.dt.float32
    mdt = getattr(mybir.dt, MM_DT)
    Exp = mybir.ActivationFunctionType.Exp
    Copy = mybir.ActivationFunctionType.Copy
    is_ge = mybir.AluOpType.is_ge

    def c(ap):
        return ap

    nc = bacc.Bacc("TRN2", target_bir_lowering=False, debug=False, num_devices=8)
    xT = nc.dram_tensor("xT", [C, T], mdt, kind="ExternalInput").ap()
    wqkT = nc.dram_tensor("wqkT", [C, 2 * LC], mdt, kind="ExternalInput").ap()
    bqk = nc.dram_tensor("bqk", [P, 2 * LC // P], f32, kind="ExternalInput").ap()
    wvT = nc.dram_tensor("wvT", [C, LC], mdt, kind="ExternalInput").ap()
    wpT = nc.dram_tensor("wpT", [LC, C], mdt, kind="ExternalInput").ap()
    bpj = nc.dram_tensor("bpj", [P, C // P], f32, kind="ExternalInput").ap()
    zT = nc.dram_tensor("zT", [C, T], f32, kind="ExternalOutput").ap()

    with tile.TileContext(nc) as tc:
        with ExitStack() as outer:
            persist = outer.enter_context(tc.tile_pool(name="persist", bufs=1))
            # qk_sb: out-ch blocks 0-3 = q, 4-7 = k; [out-ch 128, tok 2048]
            qk_sb = [persist.tile([P, T], mdt, tag=f"qk{i}", name=f"qk{i}") for i in range(8)]
            # v_sb[kb]: [tok 128, head 8, d 64 + ones col]
            v_sb = [persist.tile([P, NH, D + 1], mdt, tag=f"v{i}", name=f"v{i}") for i in range(NKB)]
            bqk_sb = persist.tile([P, 8], f32, tag="bqk")
            bpj_sb = persist.tile([P, 8], f32, tag="bpj")
            nc.sync.dma_start(bqk_sb[:], bqk)
            nc.sync.dma_start(bpj_sb[:], bpj)

            # ---- Stage A/B: qk projection + v projection, streaming x ----
            with tc.tile_pool(name="wts", bufs=1) as wpool, \
                 tc.tile_pool(name="xs", bufs=2) as xpool, \
                 tc.tile_pool(name="psab", bufs=4, space="PSUM") as pspool:
                wqk_sb = [wpool.tile([P, 2 * LC], mdt, tag=f"wqk{i}", name=f"wqk{i}") for i in range(IC)]
                wv_sb = [wpool.tile([P, LC], mdt, tag=f"wv{i}", name=f"wv{i}") for i in range(IC)]
                for i in range(IC):
                    nc.sync.dma_start(wqk_sb[i][:], wqkT[i * P:(i + 1) * P, :])
                    nc.sync.dma_start(wv_sb[i][:], wvT[i * P:(i + 1) * P, :])
                for tt in range(NQT):
                    xt = [xpool.tile([P, QT], mdt, tag=f"x{i}", name=f"x{i}") for i in range(IC)]
                    for i in range(IC):
                        nc.sync.dma_start(
                            xt[i][:], xT[i * P:(i + 1) * P, tt * QT:(tt + 1) * QT])
                    # qk-proj: psum[out-ch 128, tok 512] accumulated over in-ch
                    for oc in range(8):
                        ps = pspool.tile([P, QT], f32, tag="psA")
                        for i in range(IC):
                            nc.tensor.matmul(
                                ps[:], c(wqk_sb[i][:, oc * P:(oc + 1) * P]),
                                c(xt[i][:]), start=(i == 0), stop=(i == IC - 1))
                        nc.vector.tensor_scalar_add(
                            qk_sb[oc][:, tt * QT:(tt + 1) * QT], ps[:],
                            bqk_sb[:, oc:oc + 1])
                    # v-proj: psum[tok 128, out-ch 512] per tok block
                    for tb in range(4):
                        kb = tt * 4 + tb
                        ps = pspool.tile([P, NH, D], f32, tag="psB")
                        for i in range(IC):
                            nc.tensor.matmul(
                                ps[:], c(xt[i][:, tb * P:(tb + 1) * P]),
                                c(wv_sb[i][:]), start=(i == 0), stop=(i == IC - 1))
                        nc.scalar.activation(v_sb[kb][:, :, 0:D], ps[:], Copy)
                        # ones column for the softmax-denominator row of att@V
                        nc.scalar.activation(
                            v_sb[kb][:, :, D:D + 1], ps[:, :, 0:1],
                            mybir.ActivationFunctionType.Identity,
                            bias=1.0, scale=0.0)

            # ---- Stage C: attention;  Stage D: c_proj ----
            with tc.tile_pool(name="wp", bufs=1) as wppool, \
                 tc.tile_pool(name="ybuf", bufs=1) as ypool, \
                 tc.tile_pool(name="att", bufs=4) as apool, \
                 tc.tile_pool(name="pss", bufs=2, space="PSUM") as ps_s_pool, \
                 tc.tile_pool(name="pso", bufs=2, space="PSUM") as ps_o_pool, \
                 tc.tile_pool(name="nrm", bufs=4) as nrm_pool, \
                 tc.tile_pool(name="yraw", bufs=8) as yrawpool, \
                 tc.tile_pool(name="ptb", bufs=1, space="PSUM") as ps_b_pool, \
                 tc.tile_pool(name="psz", bufs=1, space="PSUM") as ps_z_pool, \
                 tc.tile_pool(name="zev", bufs=3) as zpool:
                wp_sb = [wppool.tile([P, C], mdt, tag=f"wp{i}", name=f"wp{i}") for i in range(4)]
                # y_sb: attention out, [local-ch 128, tok 2048] x 4 blocks
                y_sb = [ypool.tile([P, T], mdt, tag=f"y{i}", name=f"y{i}") for i in range(4)]
                for i in range(4):
                    nc.sync.dma_start(wp_sb[i][:], wpT[i * P:(i + 1) * P, :])
                # triangular mask (keep j >= p), shared by all diagonal blocks
                maskf = wppool.tile([P, QT], f32, tag="maskf", name="maskf")
                nc.vector.memset(maskf[:], 1.0)
                nc.gpsimd.affine_select(
                    maskf[:], maskf[:], compare_op=is_ge, fill=0.0,
                    base=0, pattern=[[1, QT]], channel_multiplier=-1)
                # all-ones column block, lhsT of the R-broadcast matmuls
                ones_sb = wppool.tile([P, D], f32, tag="ones", name="ones")
                nc.scalar.activation(
                    ones_sb[:], wp_sb[0][:, 0:D],
                    mybir.ActivationFunctionType.Identity, bias=1.0, scale=0.0)
                for qtt in range(NQT):
                    # S rows live at 32-aligned partitions (engine AP rule)
                    sgs = [nrm_pool.tile([P, QT], f32, tag=f"sg{i}", bufs=1,
                                         name=f"sg{i}") for i in range(2)]
                    rgs = [nrm_pool.tile([P, QT], f32, tag=f"rg{i}", bufs=1,
                                         name=f"rg{i}") for i in range(2)]
                    for i in range(2):
                        nc.vector.memset(sgs[i][:], 1.0)
                    yraws = []
                    for h in range(NH):
                        p0 = (h % 2) * D
                        qt_i = h // 2
                        kt_i = 4 + h // 2
                        nkb = (qtt + 1) * 4
                        po = ps_o_pool.tile([D + 1, QT], f32, tag="po")
                        # process kb in pairs sharing one PSUM tile + one exp
                        for pi in range(nkb // 2):
                            kbs = (2 * pi, 2 * pi + 1)
                            ns, c0s = [], []
                            for kb in kbs:
                                e = kb * P - qtt * QT
                                c0s.append(max(e, 0))
                                ns.append(QT - max(e, 0))
                            # pack both live column ranges into one tile; each
                            # matmul's output must stay inside one 512-col bank
                            o0 = 0
                            o1 = ns[0] if ns[0] + ns[1] <= QT else QT
                            width = o1 + ns[1]
                            ps = ps_s_pool.tile([P, 2 * QT], f32, tag="ps")
                            at = apool.tile([P, 2 * QT], mdt, tag="at")
                            for kb, n, c0, o in zip(kbs, ns, c0s, (o0, o1)):
                                nc.tensor.matmul(
                                    ps[:, o:o + n],
                                    c(qk_sb[kt_i][p0:p0 + D,
                                                  kb * P:(kb + 1) * P]),
                                    c(qk_sb[qt_i][p0:p0 + D,
                                                  qtt * QT + c0:(qtt + 1) * QT]),
                                    start=True, stop=True)
                            nc.scalar.activation(at[:, 0:width], ps[:, 0:width],
                                                 Exp, scale=0.125)
                            for kb, n, c0, o in zip(kbs, ns, c0s, (o0, o1)):
                                if kb * P - qtt * QT >= 0:
                                    # zero strict upper triangle; it never
                                    # reaches past the first 128 live columns
                                    m = min(n, P)
                                    nc.vector.tensor_mul(at[:, o:o + m],
                                                         at[:, o:o + m],
                                                         maskf[:, 0:m])
                                nc.tensor.matmul(
                                    po[:, c0:QT], c(v_sb[kb][:, h, :]),
                                    c(at[:, o:o + n]),
                                    start=(kb == 0), stop=(kb == nkb - 1))
                        # evict numerator+sums to SBUF, release the PSUM bank
                        yraw = yrawpool.tile([D + 1, QT], f32, tag="yraw")
                        nc.vector.tensor_copy(yraw[:], po[:])
                        r0 = 32 * (h % 4)
                        nc.vector.tensor_copy(sgs[h // 4][r0:r0 + 1, :],
                                              yraw[D:D + 1, :])
                        yraws.append(yraw)
                    # two reciprocals cover all 8 heads of this query tile
                    for i in range(2):
                        nc.vector.reciprocal(rgs[i][:], sgs[i][:])
                    for h in range(NH):
                        p0 = (h % 2) * D
                        r0 = 32 * (h % 4)
                        r_ap = rgs[h // 4][r0:r0 + 1, :]
                        if r0 == 96:  # matmul operands must start at 0/32/64
                            rfix = nrm_pool.tile([1, QT], f32, tag="rfix", bufs=2)
                            nc.vector.tensor_copy(rfix[:], r_ap)
                            r_ap = rfix[:]
                            r0 = 0
                        # broadcast R across 64 partitions via a K=1 matmul
                        btp = ps_b_pool.tile([D, QT], f32, tag="btp")
                        nc.tensor.matmul(
                            btp[:], ones_sb[r0:r0 + 1, 0:D],
                            r_ap, start=True, stop=True)
                        nc.vector.tensor_mul(
                            y_sb[h // 2][p0:p0 + D, qtt * QT:(qtt + 1) * QT],
                            yraws[h][0:D, :], btp[:])
                    # c_proj for this token tile becomes ready as soon as all
                    # heads of qtt are done; gives the PE gap-filler work
                    tt = qtt
                    for oc in range(8):
                        ps = ps_z_pool.tile([P, QT], f32, tag="pz")
                        for i in range(4):
                            nc.tensor.matmul(
                                ps[:], c(wp_sb[i][:, oc * P:(oc + 1) * P]),
                                c(y_sb[i][:, tt * QT:(tt + 1) * QT]),
                                start=(i == 0), stop=(i == 3))
                        zt = zpool.tile([P, QT], f32, tag="zt")
                        nc.vector.tensor_scalar_add(zt[:], ps[:], bpj_sb[:, oc:oc + 1])
                        nc.sync.dma_start(
                            zT[oc * P:(oc + 1) * P, tt * QT:(tt + 1) * QT], zt[:])
    nc.compile()
    return nc


def get_nc():
    if "nc" not in _nc_cache:
        _nc_cache["nc"] = _build_nc()
    return _nc_cache["nc"]


def _mm_np_dtype():
    if MM_DT == "bfloat16":
        import ml_dtypes
        return np.dtype(ml_dtypes.bfloat16)
    return np.dtype(np.float32)


def make_in_maps(x, Wqkv, bqkv, Wproj, bproj):
    x = np.asarray(x, np.float32)
    Wqkv = np.asarray(Wqkv, np.float32)
    bqkv = np.asarray(bqkv, np.float32)
    Wproj = np.asarray(Wproj, np.float32)
    bproj = np.asarray(bproj, np.float32)
    Wq, Wk, Wv = Wqkv[0:C], Wqkv[C:2 * C], Wqkv[2 * C:3 * C]
    bq, bk, bv = bqkv[0:C], bqkv[C:2 * C], bqkv[2 * C:3 * C]
    mdt = _mm_np_dtype()
    in_maps = []
    for b in range(B):
        xTb = np.ascontiguousarray(x[b].T.astype(mdt))
        for s in range(2):
            cols = slice(s * LC, (s + 1) * LC)
            wqkT = np.ascontiguousarray(
                np.concatenate([Wq[cols], Wk[cols]], 0).T.astype(mdt))
            bqk_ = np.concatenate([bq[cols], bk[cols]])
            wvT_ = np.ascontiguousarray(Wv[cols].T.astype(mdt))
            wpT_ = np.ascontiguousarray(Wproj[:, cols].T.astype(mdt))
            bp_eff = bv[cols] @ Wproj[:, cols].T
            if s == 0:
                bp_eff = bp_eff + bproj
            in_maps.append({
                "xT": xTb,
                "wqkT": wqkT,
                "bqk": np.ascontiguousarray(bqk_.reshape(8, P).T),
                "wvT": wvT_,
                "wpT": wpT_,
                "bpj": np.ascontiguousarray(bp_eff.astype(np.float32).reshape(8, P).T),
            })
    return in_maps


def gather_out(results):
    out = np.empty((B, T, C), np.float32)
    for b in range(B):
        zt = results[2 * b]["zT"] + results[2 * b + 1]["zT"]
        out[b] = zt.T
    return out


def kernel(x, Wqkv, bqkv, Wproj, bproj):
    from concourse.bass_utils import run_bass_kernel_spmd

    in_maps = make_in_maps(x, Wqkv, bqkv, Wproj, bproj)
    try:
        res = run_bass_kernel_spmd(get_nc(), in_maps, core_ids=list(range(8)))
    except Exception:
        # transient device faults have been observed once; retry a single time
        res = run_bass_kernel_spmd(get_nc(), in_maps, core_ids=list(range(8)))
    return gather_out(res.results)



# revision 6
# speedup vs baseline: 1.3800x; 1.3800x over previous
"""Causal self-attention (B=4, T=2048, C=1024, H=16) on 8 trn2 NeuronCores.

Sharding: core = (batch b, head-half s).  Each core computes q/k/v
projections for its 8 heads (weights pre-sliced/transposed on host),
causal attention with transposed score tiles, and a partial
(row-sharded) c_proj.  Host gather sums the two partials per batch.

v2 layout/scheduling:
  - all matmul operands bf16 (FWL weight loads, 1 cyc/row streaming)
  - score matmuls for a head PAIR run concurrently on PE row-groups
    0-63 / 64-127 (auto tile_position from base partitions)
  - one Exp activation per (pair, kb) covers both heads via a
    [128, 2, n] strided AP over a 2-bank PSUM tile
  - projections (token tile tt) and attention (query tile tt-1) are
    issued interleaved so ACT exp work overlaps PE projection work and
    the PE never idles long enough to drop the HAM clock
  - c_proj for qtt is deferred into the next attention phase as PE
    gap-filler; softmax denominators use reciprocal_approx_fast

Device data layout (biases fp32, rest bf16):
  xT    [1024, 2048]  x[b].T                      (in-ch on partitions)
  wqkT  [1024, 1024]  [Wq_local | Wk_local].T     (in-ch on partitions)
  bqk   [128, 8]      q/k bias, per out-ch block
  wvT   [1024, 512]   Wv_local.T
  wpT   [512, 1024]   Wproj[:, local].T
  bpj   [128, 8]      bproj + bv@WprojT (folded), half of it per core
  zT    [1024, 2048]  partial output, transposed (fp32)
"""

import os
import sys

sys.path.insert(0, "/opt/trn_rl_repo")

import numpy as np

B, T, C, H = 4, 2048, 1024, 16
D = 64          # head dim
NH = 8          # heads per core
LC = NH * D     # local channels = 512
P = 128
QT = 512        # query tile (also matmul moving free dim)
NQT = T // QT   # 4
NKB = T // P    # 16 key blocks
IC = C // P     # 8 input-channel blocks

MM_DT = os.environ.get("BASS_ATTN_MM_DT", "bfloat16")

_nc_cache = {}


def _build_nc():
    from contextlib import ExitStack

    import concourse.bass as bass  # noqa: F401
    import concourse.mybir as mybir
    from concourse import bacc, tile

    f32 = mybir.dt.float32
    mdt = getattr(mybir.dt, MM_DT)
    Exp = mybir.ActivationFunctionType.Exp
    Copy = mybir.ActivationFunctionType.Copy
    Ident = mybir.ActivationFunctionType.Identity
    is_ge = mybir.AluOpType.is_ge

    nc = bacc.Bacc("TRN2", target_bir_lowering=False, debug=False, num_devices=8)
    xT = nc.dram_tensor("xT", [C, T], mdt, kind="ExternalInput").ap()
    wqkT = nc.dram_tensor("wqkT", [C, 2 * LC], mdt, kind="ExternalInput").ap()
    bqk = nc.dram_tensor("bqk", [P, 2 * LC // P], f32, kind="ExternalInput").ap()
    wvT = nc.dram_tensor("wvT", [C, LC], mdt, kind="ExternalInput").ap()
    wpT = nc.dram_tensor("wpT", [LC, C], mdt, kind="ExternalInput").ap()
    bpj = nc.dram_tensor("bpj", [P, C // P], f32, kind="ExternalInput").ap()
    zT = nc.dram_tensor("zT", [C, T], f32, kind="ExternalOutput").ap()

    with tile.TileContext(nc) as tc:
        with ExitStack() as stk:
            persist = stk.enter_context(tc.tile_pool(name="persist", bufs=1))
            # qk_sb: out-ch blocks 0-3 = q, 4-7 = k; [out-ch 128, tok 2048]
            qk_sb = [persist.tile([P, T], mdt, tag=f"qk{i}", name=f"qk{i}")
                     for i in range(8)]
            # v_sb[kb]: [tok 128, head 8, d 64 + ones col]
            v_sb = [persist.tile([P, NH, D + 1], mdt, tag=f"v{i}", name=f"v{i}")
                    for i in range(NKB)]
            # y_sb: attention out, [local-ch 128, tok 2048] x 4 blocks
            y_sb = [persist.tile([P, T], mdt, tag=f"y{i}", name=f"y{i}")
                    for i in range(4)]
            wqk_sb = [persist.tile([P, 2 * LC], mdt, tag=f"wqk{i}", name=f"wqk{i}")
                      for i in range(IC)]
            wv_sb = [persist.tile([P, LC], mdt, tag=f"wv{i}", name=f"wv{i}")
                     for i in range(IC)]
            wp_sb = [persist.tile([P, C], mdt, tag=f"wp{i}", name=f"wp{i}")
                     for i in range(4)]
            bqk_sb = persist.tile([P, 8], f32, tag="bqk")
            bpj_sb = persist.tile([P, 8], f32, tag="bpj")
            maskf = persist.tile([P, P], mdt, tag="maskf")
            # all-ones column block, lhsT of the K=1 R-broadcast matmuls
            onesc = persist.tile([P, D], mdt, tag="onesc")

            nc.sync.dma_start(bqk_sb[:], bqk)
            nc.sync.dma_start(bpj_sb[:], bpj)

            xpool = stk.enter_context(tc.tile_pool(name="xs", bufs=2))
            atpool = stk.enter_context(tc.tile_pool(name="at", bufs=3))
            yrawp = stk.enter_context(tc.tile_pool(name="yraw", bufs=3))
            nrmp = stk.enter_context(tc.tile_pool(name="nrm", bufs=2))
            ztp = stk.enter_context(tc.tile_pool(name="zt", bufs=3))
            # PSUM: acc 2 banks + pss 2x2 banks + po 2 banks = 8
            accp = stk.enter_context(tc.tile_pool(name="acc", bufs=2, space="PSUM"))
            pssp = stk.enter_context(tc.tile_pool(name="pss", bufs=2, space="PSUM"))
            pop = stk.enter_context(tc.tile_pool(name="po", bufs=1, space="PSUM"))

            # triangular mask (keep j >= p) and the bcast ones pattern
            nc.vector.memset(maskf[:], 1.0)
            nc.gpsimd.affine_select(
                maskf[:], maskf[:], compare_op=is_ge, fill=0.0,
                base=0, pattern=[[1, P]], channel_multiplier=-1)
            nc.vector.memset(onesc[:], 1.0)

            # x tiles for token tile tt (8 in-ch blocks)
            def dma_x(tt):
                xt = [xpool.tile([P, QT], mdt, tag=f"x{i}", name=f"x{i}")
                      for i in range(IC)]
                for i in range(IC):
                    nc.sync.dma_start(
                        xt[i][:], xT[i * P:(i + 1) * P, tt * QT:(tt + 1) * QT])
                return xt

            # initial DMAs: interleave wqk with x(0) so the first chain
            # starts early; wv/wp after
            xt0 = [xpool.tile([P, QT], mdt, tag=f"x{i}", name=f"x{i}")
                   for i in range(IC)]
            for i in range(IC):
                nc.sync.dma_start(wqk_sb[i][:], wqkT[i * P:(i + 1) * P, :])
                nc.sync.dma_start(
                    xt0[i][:], xT[i * P:(i + 1) * P, 0:QT])
            for i in range(IC):
                nc.sync.dma_start(wv_sb[i][:], wvT[i * P:(i + 1) * P, :])
            for i in range(4):
                nc.sync.dma_start(wp_sb[i][:], wpT[i * P:(i + 1) * P, :])

            # ---------------- proj chains (A-list) ----------------
            def qk_chain(xt, tt, oc):
                ps = accp.tile([P, QT], f32, tag="acc", name="psA")
                for i in range(IC):
                    nc.tensor.matmul(
                        ps[:], wqk_sb[i][:, oc * P:(oc + 1) * P], xt[i][:],
                        start=(i == 0), stop=(i == IC - 1))
                nc.vector.tensor_scalar_add(
                    qk_sb[oc][:, tt * QT:(tt + 1) * QT], ps[:],
                    bqk_sb[:, oc:oc + 1])

            def v_chain(xt, tt, tb):
                kb = tt * 4 + tb
                ps = accp.tile([P, NH, D], f32, tag="acc", name="psB")
                for i in range(IC):
                    nc.tensor.matmul(
                        ps[:], xt[i][:, tb * P:(tb + 1) * P], wv_sb[i][:],
                        start=(i == 0), stop=(i == IC - 1))
                nc.scalar.activation(v_sb[kb][:, :, 0:D], ps[:], Copy)
                nc.scalar.activation(
                    v_sb[kb][:, :, D:D + 1], ps[:, :, 0:1],
                    Ident, bias=1.0, scale=0.0)

            def proj_list(tt):
                xt = dma_x(tt) if tt > 0 else xt0
                ops = []
                for oc in range(8):
                    ops.append(lambda oc=oc: qk_chain(xt, tt, oc))
                for tb in range(4):
                    ops.append(lambda tb=tb: v_chain(xt, tt, tb))
                return ops

            # ---------------- attention (B-list) ----------------
            def cproj_chain(tt, oc):
                ps = accp.tile([P, QT], f32, tag="acc", name="pz")
                for i in range(4):
                    nc.tensor.matmul(
                        ps[:], wp_sb[i][:, oc * P:(oc + 1) * P],
                        y_sb[i][:, tt * QT:(tt + 1) * QT],
                        start=(i == 0), stop=(i == 3))
                zt = ztp.tile([P, QT], f32, tag="zt")
                nc.vector.tensor_scalar_add(zt[:], ps[:], bpj_sb[:, oc:oc + 1])
                nc.sync.dma_start(
                    zT[oc * P:(oc + 1) * P, tt * QT:(tt + 1) * QT], zt[:])

            def att_list(qtt, deferred):
                """Emit ops for attention over query tile qtt.  `deferred`
                is a list of closures (prev qtt's c_proj) used as PE
                gap-filler around the normalization."""
                nkb = (qtt + 1) * 4
                # per-pair state
                sgs = [nrmp.tile([P, QT], f32, tag=f"sg{i}", name=f"sg{i}")
                       for i in range(2)]
                rgs = [nrmp.tile([P, QT], f32, tag=f"rg{i}", name=f"rg{i}")
                       for i in range(2)]
                rgb = [nrmp.tile([P, QT], mdt, tag=f"rb{i}", name=f"rb{i}")
                       for i in range(2)]
                steps = [(pr, kb) for pr in range(4) for kb in range(nkb)]
                ps_t = {}
                at_t = {}
                po_t = {}
                yraw = {}

                def s_stage(k):
                    pr, kb = steps[k]
                    a = 2 * pr
                    qt_i, kt_i = pr, 4 + pr
                    if kb == 0:
                        po_t[pr] = pop.tile([D + 1, 2, QT], f32, tag="po",
                                            name="po")
                    e = kb * P - qtt * QT
                    c0 = max(e, 0)
                    n = QT - c0
                    ps = pssp.tile([P, 2, QT], f32, tag="ps", name="ps")
                    ps_t[k] = (ps, c0, n)
                    for s in range(2):
                        p0 = s * D
                        nc.tensor.matmul(
                            ps[:, s, 0:n],
                            qk_sb[kt_i][p0:p0 + D, kb * P:(kb + 1) * P],
                            qk_sb[qt_i][p0:p0 + D,
                                        qtt * QT + c0:(qtt + 1) * QT],
                            start=True, stop=True)

                def sx_stage(k):
                    pr, kb = steps[k]
                    ps, c0, n = ps_t[k]
                    at = atpool.tile([P, 2, QT], mdt, tag="at", name="at")
                    at_t[k] = at
                    nc.scalar.activation(at[:, :, 0:n], ps[:, :, 0:n],
                                         Exp, scale=0.125)
                    if kb >= qtt * 4:  # diagonal block: zero upper triangle
                        for s in range(2):
                            nc.vector.tensor_mul(at[:, s, 0:P],
                                                 at[:, s, 0:P], maskf[:])
                    po = po_t[pr]
                    for s in range(2):
                        h = 2 * pr + s
                        nc.tensor.matmul(
                            po[:, s, c0:QT], v_sb[kb][:, h, :],
                            at[:, s, 0:n],
                            start=(kb == 0), stop=(kb == nkb - 1))
                    if kb == nkb - 1:
                        # evict numerator+sums, release the po bank pair
                        yr = yrawp.tile([D + 1, 2, QT], mdt, tag="yraw",
                                        name="yr")
                        yraw[pr] = yr
                        nc.vector.tensor_copy(yr[:], po[:])
                        r0 = 64 * (pr % 2)
                        g = pr // 2
                        for s in range(2):
                            nc.vector.tensor_copy(
                                sgs[g][r0 + 32 * s:r0 + 32 * s + 1, :],
                                po[D:D + 1, s, :])

                def norm_pairs(g):
                    # reciprocal for sgs group g (pairs 2g, 2g+1), then
                    # broadcast + normalize those 4 heads.  Unused rows hold
                    # garbage; only rows 0/32/64/96 are ever consumed.
                    nc.vector.reciprocal_approx_fast(rgs[g][:], sgs[g][:])
                    nc.vector.tensor_copy(rgb[g][:], rgs[g][:])
                    for pr in (2 * g, 2 * g + 1):
                        btp = accp.tile([P, QT], f32, tag="acc", name="btp")
                        for s in range(2):
                            r = 64 * (pr % 2) + 32 * s
                            r_ap = rgb[g][r:r + 1, :]
                            if r == 96:  # matmul operands must start at 0/32/64
                                rfx = nrmp.tile([1, QT], mdt, tag="rfx",
                                                bufs=2, name="rfx")
                                nc.vector.tensor_copy(rfx[:], r_ap)
                                r_ap = rfx[:]
                                r = 0
                            nc.tensor.matmul(
                                btp[s * D:(s + 1) * D, :],
                                onesc[r:r + 1, :], r_ap,
                                start=True, stop=True)
                        yr = yraw[pr]
                        for s in range(2):
                            p0 = s * D
                            nc.vector.tensor_mul(
                                y_sb[pr][p0:p0 + D, qtt * QT:(qtt + 1) * QT],
                                yr[0:D, s, :], btp[p0:p0 + D, :])

                ops = []
                nsteps = len(steps)
                ops.append(lambda: s_stage(0))
                for k in range(nsteps):
                    if k + 1 < nsteps:
                        ops.append(lambda k=k: s_stage(k + 1))
                    ops.append(lambda k=k: sx_stage(k))
                    pr, kb = steps[k]
                    if kb == nkb - 1 and pr in (1, 3):
                        ops.append(lambda g=pr // 2: norm_pairs(g))
                        if pr == 1:
                            # fill PE while pairs 2,3 run: deferred c_proj
                            ops.extend(deferred)
                            deferred = []
                ops.extend(deferred)
                return ops

            def interleave(a, b):
                """Merge op lists evenly (a paced across b)."""
                out = []
                na, nb = len(a), len(b)
                ia = ib = 0
                tot = na + nb
                for i in range(tot):
                    if ia * tot <= i * na and ia < na:
                        out.append(a[ia]); ia += 1
                    elif ib < nb:
                        out.append(b[ib]); ib += 1
                    else:
                        out.append(a[ia]); ia += 1
                return out

            # ---------------- schedule ----------------
            for op in proj_list(0):
                op()
            deferred = []
            for t in range(1, NQT + 1):
                att = att_list(t - 1, deferred)
                deferred = [lambda oc=oc, t=t: cproj_chain(t - 1, oc)
                            for oc in range(8)]
                if t < NQT:
                    sched = interleave(proj_list(t), att)
                else:
                    sched = att
                for op in sched:
                    op()
            for op in deferred:
                op()
    nc.compile()
    return nc


def get_nc():
    if "nc" not in _nc_cache:
        _nc_cache["nc"] = _build_nc()
    return _nc_cache["nc"]


def _mm_np_dtype():
    if MM_DT == "bfloat16":
        import ml_dtypes
        return np.dtype(ml_dtypes.bfloat16)
    return np.dtype(np.float32)


def make_in_maps(x, Wqkv, bqkv, Wproj, bproj):
    x = np.asarray(x, np.float32)
    Wqkv = np.asarray(Wqkv, np.float32)
    bqkv = np.asarray(bqkv, np.float32)
    Wproj = np.asarray(Wproj, np.float32)
    bproj = np.asarray(bproj, np.float32)
    Wq, Wk, Wv = Wqkv[0:C], Wqkv[C:2 * C], Wqkv[2 * C:3 * C]
    bq, bk, bv = bqkv[0:C], bqkv[C:2 * C], bqkv[2 * C:3 * C]
    mdt = _mm_np_dtype()
    in_maps = []
    for b in range(B):
        xTb = np.ascontiguousarray(x[b].T.astype(mdt))
        for s in range(2):
            cols = slice(s * LC, (s + 1) * LC)
            wqkT = np.ascontiguousarray(
                np.concatenate([Wq[cols], Wk[cols]], 0).T.astype(mdt))
            bqk_ = np.concatenate([bq[cols], bk[cols]])
            wvT_ = np.ascontiguousarray(Wv[cols].T.astype(mdt))
            wpT_ = np.ascontiguousarray(Wproj[:, cols].T.astype(mdt))
            bp_eff = bv[cols] @ Wproj[:, cols].T
            if s == 0:
                bp_eff = bp_eff + bproj
            in_maps.append({
                "xT": xTb,
                "wqkT": wqkT,
                "bqk": np.ascontiguousarray(bqk_.reshape(8, P).T),
                "wvT": wvT_,
                "wpT": wpT_,
                "bpj": np.ascontiguousarray(
                    bp_eff.astype(np.float32).reshape(8, P).T),
            })
    return in_maps


def gather_out(results):
    out = np.empty((B, T, C), np.float32)
    for b in range(B):
        zt = results[2 * b]["zT"] + results[2 * b + 1]["zT"]
        out[b] = zt.T
    return out


def kernel(x, Wqkv, bqkv, Wproj, bproj):
    from concourse.bass_utils import run_bass_kernel_spmd

    in_maps = make_in_maps(x, Wqkv, bqkv, Wproj, bproj)
    try:
        res = run_bass_kernel_spmd(get_nc(), in_maps, core_ids=list(range(8)))
    except Exception:
        # transient device faults have been observed once; retry a single time
        res = run_bass_kernel_spmd(get_nc(), in_maps, core_ids=list(range(8)))
    return gather_out(res.results)
